# revision 1
# baseline (speedup 1.0000x reference)
"""CrossModalMDTA Trainium2 kernel (8-core data-parallel over batch).

Per-core pipeline (one batch sample, C=192, H=W=128, 4 heads, head_dim=48):
  q  = dw3x3(conv1x1(f_opt, w_q), w_qdw)            [C, N]
  kv = dw3x3(conv1x1(f_sar, w_kv), w_kvdw)          [2C, N]
  G  = (q/|q|) @ (k/|k|)^T per head  (48x48)        l2 norms applied to the
                                                    Gram matrix, not the tensors
  out = w_out @ (softmax(G*temp) @ v)               [C, N]

Layout: channels on partitions, flattened spatial on the free dim with a
4-column pad per image row ([C, 128, 132], valid w in 2..130) so the nine
depthwise taps are free-dim-shifted multiply-accumulates with correct zero
padding.  1x1 convs / attention / output projection run on TensorE in bf16
with fp32 PSUM accumulation.  The depthwise conv is split between TensorE
(diag-weight matmuls whose shifted reads come from the rhs access pattern,
accumulated in PSUM) and VectorE (tensor_scalar product at 4x + tensor_tensor
add at 2x).  The Gram contraction gets its [N, C] operands from batched xbar
DMA transposes (one instruction per band transposes 16 128x128 blocks).
v's depthwise output round-trips through DRAM between phases to fit SBUF.
"""

import numpy as np
import ml_dtypes
from contextlib import ExitStack

import concourse.bass as bass
import concourse.mybir as mybir
import concourse.tile as tile
from concourse import bacc
from concourse.bass_utils import run_bass_kernel_spmd
from concourse.masks import make_identity

BF = mybir.dt.bfloat16
F8 = mybir.dt.float8e4
F32 = mybir.dt.float32
DR = mybir.MatmulPerfMode.DoubleRow
SCL = 64.0
ALU = mybir.AluOpType
ACT = mybir.ActivationFunctionType

B = 8
C = 192
HH = 128
WW = 128
NH = 4
HD = 48
N = HH * WW            # 16384
WP = WW + 4            # 132 padded row width (2 guard cols each side)
NP = HH * WP           # 16896
BAND = 16              # h-rows per band
NB = HH // BAND        # 8 bands
BN = BAND * WW         # 2048 valid elems per band
BROWS = BAND + 2       # band buffer rows (1-row halo each side)
BBUF = BROWS * WP      # 2376
NSL = BN // 512        # 512-wide psum slices per band

# which depthwise blocks go on the PE (diag matmuls) per band; the rest go on
# the DVE (tensor_scalar + tensor_tensor).  Tunable balance knob.
PE_DW = {
    "q": [False] * NB,
    "k": [False] * NB,
    "qk": [True] * NB,
    "va": [True] * NB,
    "vb": [True] * NB,
}

_NC_CACHE = {}


def _taps():
    # (tap index, dh, dw) for the 3x3 correlation; center first so it can
    # initialize the accumulator (never range-restricted, always full-size).
    out = [(4, 0, 0)]
    for ky in range(3):
        for kx in range(3):
            t = ky * 3 + kx
            if t != 4:
                out.append((t, ky - 1, kx - 1))
    return out


def build_nc():
    nc = bacc.Bacc("TRN2", target_bir_lowering=False, debug=False, num_devices=B)

    xo_d = nc.dram_tensor("x_opt", [C, N], BF, kind="ExternalInput").ap()
    xs_d = nc.dram_tensor("x_sar", [C, N], BF, kind="ExternalInput").ap()
    xo8_d = nc.dram_tensor("x_opt8", [96, 2 * N], F8, kind="ExternalInput").ap()
    xr8_d = nc.dram_tensor("x_sar8", [96, 2 * N], F8, kind="ExternalInput").ap()
    wq8_d = nc.dram_tensor("w_q8", [96, 2 * 128], F8, kind="ExternalInput").ap()
    wqh8_d = nc.dram_tensor("w_qh8", [96, 2 * 64], F8, kind="ExternalInput").ap()
    wk8_d = nc.dram_tensor("w_k8", [96, 2 * 128], F8, kind="ExternalInput").ap()
    wkh8_d = nc.dram_tensor("w_kh8", [96, 2 * 64], F8, kind="ExternalInput").ap()
    wvT_d = nc.dram_tensor("w_v_t", [C, C], BF, kind="ExternalInput").ap()
    woT_d = nc.dram_tensor("w_o_t", [C, C], BF, kind="ExternalInput").ap()
    dwq_d = nc.dram_tensor("dw_q", [C, 9], F32, kind="ExternalInput").ap()
    dwk_d = nc.dram_tensor("dw_k", [C, 9], F32, kind="ExternalInput").ap()
    dwv_d = nc.dram_tensor("dw_v", [C, 9], F32, kind="ExternalInput").ap()
    temp_d = nc.dram_tensor("temp", [1, NH], F32, kind="ExternalInput").ap()
    out_d = nc.dram_tensor("out", [C, N], F32, kind="ExternalOutput").ap()

    with ExitStack() as ctx:
        tc = ctx.enter_context(tile.TileContext(nc))
        consts = ctx.enter_context(tc.tile_pool(name="consts", bufs=1))
        small = ctx.enter_context(tc.tile_pool(name="small", bufs=1))
        gram_ps = ctx.enter_context(tc.tile_pool(name="gram_ps", bufs=1, space="PSUM"))
        dram = ctx.enter_context(tc.tile_pool(name="dram", bufs=1, space="DRAM"))

        # ---- weights ----
        wq8 = consts.tile([96, 2, 128], F8, tag="wq8")
        wqh8 = consts.tile([96, 2, 64], F8, tag="wqh8")
        wk8 = consts.tile([96, 2, 128], F8, tag="wk8")
        wkh8 = consts.tile([96, 2, 64], F8, tag="wkh8")
        nc.sync.dma_start(wq8, wq8_d.rearrange("k (j m) -> k j m", j=2))
        nc.sync.dma_start(wqh8, wqh8_d.rearrange("k (j m) -> k j m", j=2))
        nc.sync.dma_start(wk8, wk8_d.rearrange("k (j m) -> k j m", j=2))
        nc.sync.dma_start(wkh8, wkh8_d.rearrange("k (j m) -> k j m", j=2))
        wvT_a = consts.tile([128, C], BF, tag="wva")
        wvT_b = consts.tile([64, C], BF, tag="wvb")
        woT_a = consts.tile([96, C], BF, tag="woa")
        woT_b = consts.tile([96, C], BF, tag="wob")
        nc.sync.dma_start(wvT_a, wvT_d[0:128, :])
        nc.sync.dma_start(wvT_b, wvT_d[128:192, :])
        nc.sync.dma_start(woT_a, woT_d[0:96, :])
        nc.sync.dma_start(woT_b, woT_d[96:192, :])

        # depthwise weights as per-partition scalars (fp32 for DVE scalar ops,
        # bf16 for building the PE diag matrices)
        dwq0 = consts.tile([128, 9], F32, tag="dwq0")      # q channels 0:128
        dwk0 = consts.tile([128, 9], F32, tag="dwk0")      # k channels 0:128
        dwqk1 = consts.tile([128, 9], F32, tag="dwqk1")    # q 128:192 | k 128:192
        dwva = consts.tile([96, 9], F32, tag="dwva")       # v channels 0:96
        dwvb = consts.tile([96, 9], F32, tag="dwvb")       # v channels 96:192
        nc.sync.dma_start(dwq0, dwq_d[0:128, :])
        nc.sync.dma_start(dwk0, dwk_d[0:128, :])
        nc.sync.dma_start(dwqk1[0:64, :], dwq_d[128:192, :])
        nc.sync.dma_start(dwqk1[64:128, :], dwk_d[128:192, :])
        nc.sync.dma_start(dwva, dwv_d[0:96, :])
        nc.sync.dma_start(dwvb, dwv_d[96:192, :])

        ident_bf = consts.tile([HD, HD], BF, tag="idbf")
        make_identity(nc, ident_bf)
        ident_f32 = consts.tile([HD, HD], F32, tag="idf32")
        make_identity(nc, ident_f32)
        id128 = consts.tile([128, 128], BF, tag="id128")
        make_identity(nc, id128)
        id96 = consts.tile([96, 96], BF, tag="id96")
        make_identity(nc, id96)

        # PE depthwise diag matrices: diag(w_t) = identity * w[:, t]
        dgs_all = {}
        for kname, wsrc, idm, pp in (("q", dwq0, id128, 128),
                                     ("k", dwk0, id128, 128),
                                     ("qk", dwqk1, id128, 128),
                                     ("va", dwva, id96, 96),
                                     ("vb", dwvb, id96, 96)):
            lst = []
            for t in range(9):
                d = consts.tile([pp, pp], BF, tag=f"dg{kname}{t}")
                nc.vector.tensor_scalar_mul(d, idm, wsrc[:, t:t + 1])
                lst.append(d)
            dgs_all[kname] = lst

        # norm^2 accumulators (one column per band)
        n2q0 = small.tile([128, NB], F32, tag="n2q0")
        n2k0 = small.tile([128, NB], F32, tag="n2k0")
        n2qk1 = small.tile([128, NB], F32, tag="n2qk1")

        g_ps = gram_ps.tile([HD, NH * HD], F32, tag="gps")

        vdw_dram_a = dram.tile([96, N], BF, tag="vdwa")
        vdw_dram_b = dram.tile([96, N], BF, tag="vdwb")

        # =========================== PHASE A ===========================
        with ExitStack() as ctxa:
            xband = ctxa.enter_context(tc.tile_pool(name="xband", bufs=2))
            pwband = ctxa.enter_context(tc.tile_pool(name="pwband", bufs=2))
            xsband = ctxa.enter_context(tc.tile_pool(name="xsband", bufs=3))
            dwband = ctxa.enter_context(tc.tile_pool(name="dwband", bufs=2))
            qtp = ctxa.enter_context(tc.tile_pool(name="qtp", bufs=2))
            sinkp = ctxa.enter_context(tc.tile_pool(name="sinkp", bufs=3))
            ps = ctxa.enter_context(tc.tile_pool(name="ps", bufs=4, space="PSUM"))
            psd = ctxa.enter_context(tc.tile_pool(name="psd", bufs=2, space="PSUM"))

            BKEYS = ("q", "k", "qk", "va", "vb")

            def pw_band(i):
                n0 = i * BN
                xr0 = xband.tile([128, BN], BF, tag="xr0")
                xr1 = xband.tile([64, BN], BF, tag="x1")
                xo8 = xband.tile([96, 2, BN], F8, tag="xo8")
                xr8 = xband.tile([96, 2, BN], F8, tag="xr8")
                nc.gpsimd.dma_start(xr0, xs_d[0:128, n0:n0 + BN])
                nc.gpsimd.dma_start(xr1, xs_d[128:192, n0:n0 + BN])
                for j in range(2):
                    nc.gpsimd.dma_start(xo8[:, j, :],
                                        xo8_d[:, j * N + n0: j * N + n0 + BN])
                    nc.gpsimd.dma_start(xr8[:, j, :],
                                        xr8_d[:, j * N + n0: j * N + n0 + BN])

                tiles = {}
                for key in BKEYS:
                    p = 128 if key in ("q", "k", "qk") else 96
                    t = pwband.tile([p, BBUF], BF, tag=f"pw_{key}")
                    tiles[key] = t
                    t3 = t.rearrange("p (h w) -> p h w", w=WP)
                    nc.gpsimd.memset(t3[:, :, 0:2], 0.0)
                    nc.gpsimd.memset(t3[:, :, 130:132], 0.0)
                    if i == 0:
                        nc.gpsimd.memset(t3[:, 0:1, :], 0.0)

                for j in range(NSL):
                    sl = slice(j * 512, j * 512 + 512)
                    r0 = 1 + 4 * j          # band-buffer row of this psum slice

                    mm = [
                        ("q", 128, "dr", wq8, xo8),
                        ("qk", 64, "dr", wqh8, xo8),
                        ("k", 128, "dr", wk8, xr8),
                        ("qk2", 64, "dr", wkh8, xr8),
                        ("va", 96, "bf", wvT_a[:, 0:96], wvT_b[:, 0:96]),
                        ("vb", 96, "bf", wvT_a[:, 96:192], wvT_b[:, 96:192]),
                    ]
                    for name, pp, kind, la, lb in mm:
                        pt = ps.tile([pp, 512], F32, tag="pw")
                        if kind == "dr":
                            nc.tensor.matmul(pt, la, lb[:, :, sl], start=True,
                                             stop=True, perf_mode=DR)
                        else:
                            nc.tensor.matmul(pt, la, xr0[:, sl], start=True, stop=False)
                            nc.tensor.matmul(pt, lb, xr1[:, sl], start=False, stop=True)
                        pview = pt.rearrange("p (r w) -> p r w", w=WW)
                        if name == "qk":
                            dst = tiles["qk"].rearrange("p (h w) -> p h w", w=WP)
                            nc.scalar.copy(dst[0:64, r0:r0 + 4, 2:130], pview)
                        elif name == "qk2":
                            dst = tiles["qk"].rearrange("p (h w) -> p h w", w=WP)
                            nc.scalar.copy(dst[64:128, r0:r0 + 4, 2:130], pview)
                        else:
                            dst = tiles[name].rearrange("p (h w) -> p h w", w=WP)
                            nc.scalar.copy(dst[:, r0:r0 + 4, 2:130], pview)
                return tiles

            def halo_exchange(prev, cur):
                # prev row 17 <- cur row 1 ; cur row 0 <- prev row 16
                for key in BKEYS:
                    p3 = prev[key].rearrange("p (h w) -> p h w", w=WP)
                    c3 = cur[key].rearrange("p (h w) -> p h w", w=WP)
                    nc.vector.tensor_copy(p3[:, BAND + 1:BAND + 2, :], c3[:, 1:2, :])
                    nc.vector.tensor_copy(c3[:, 0:1, :], p3[:, BAND:BAND + 1, :])

            def dw_dve(src, wtile, dst, parts):
                # tensor_scalar product (4x) + tensor_tensor add (2x)
                xs = xsband.tile([parts, BBUF], BF, tag="xs")
                nc.vector.tensor_copy(xs[:, 0:BBUF - 2], src[:, 1:BBUF - 1])
                dst3 = dst.rearrange("p (r w) -> p r w", w=WW)
                s3 = src.rearrange("p (h w) -> p h w", w=WP)
                x3 = xs.rearrange("p (h w) -> p h w", w=WP)
                for t, dh, dw in _taps():
                    br = 1 + dh
                    if dw == 0:
                        insl = s3[:, br:br + BAND, 2:130]
                    elif dw == 1:
                        insl = x3[:, br:br + BAND, 2:130]
                    else:
                        insl = x3[:, br:br + BAND, 0:128]
                    if t == 4:
                        nc.vector.tensor_scalar_mul(dst3, insl, wtile[:, t:t + 1])
                    else:
                        p = sinkp.tile([parts, BAND * WW], BF, tag="prod")
                        p3 = p.rearrange("p (r w) -> p r w", w=WW)
                        nc.vector.tensor_scalar_mul(p3, insl, wtile[:, t:t + 1])
                        nc.vector.tensor_add(dst, dst, p)
                return xs

            def dw_pe(src, dgs, dst, parts):
                # diag(w_t) matmuls, shifts via the rhs access pattern,
                # accumulated in PSUM; center tap first (start=True)
                s3 = src.rearrange("p (h w) -> p h w", w=WP)
                for j in range(NSL):
                    pt = psd.tile([parts, 512], F32, tag="dw")
                    r0 = 1 + 4 * j
                    for t, dh, dw in _taps():
                        br = r0 + dh
                        if dw == 0:
                            rhs = s3[:, br:br + 4, 2:130]
                        elif dw == 1:
                            rhs = s3[:, br:br + 4, 3:131]
                        else:
                            rhs = s3[:, br:br + 4, 1:129]
                        nc.tensor.matmul(pt, dgs[t], rhs, start=(t == 4),
                                         stop=(t == 8), skip_group_check=True)
                    nc.scalar.copy(dst[:, j * 512:(j + 1) * 512], pt)

            def dw_gram_band(i, tiles):
                dws = {}
                sinks = {}
                for key, wf, parts in (("q", dwq0, 128), ("k", dwk0, 128),
                                       ("qk", dwqk1, 128), ("va", dwva, 96),
                                       ("vb", dwvb, 96)):
                    dst = dwband.tile([parts, BN], BF, tag=f"dw_{key}")
                    if PE_DW[key][i]:
                        dw_pe(tiles[key], dgs_all[key], dst, parts)
                    else:
                        sinks[key] = dw_dve(tiles[key], wf, dst, parts)
                    dws[key] = dst

                # spill v depthwise output to DRAM for phase B
                nc.gpsimd.dma_start(vdw_dram_a[:, i * BN:(i + 1) * BN], dws["va"])
                nc.gpsimd.dma_start(vdw_dram_b[:, i * BN:(i + 1) * BN], dws["vb"])

                # channel norms (sum of squares) for q and k
                for key, acc in (("q", n2q0), ("k", n2k0), ("qk", n2qk1)):
                    sink = sinks.get(key)
                    if sink is None:
                        sink = sinkp.tile([128, BN], BF, tag="nsink")
                    else:
                        sink = sink[:, 0:BN]
                    nc.scalar.activation(sink, dws[key], ACT.Square,
                                         accum_out=acc[:, i:i + 1])

                # batched transposes: one inst flips 16 128x128 blocks
                qT = qtp.tile([128, BAND, C], BF, tag="qT")
                kT = qtp.tile([128, BAND, C], BF, tag="kT")
                nc.sync.dma_start(qT[:, :, 0:128], dws["q"], transpose=True)
                nc.sync.dma_start(qT[:, :, 128:192], dws["qk"][0:64, :], transpose=True)
                nc.sync.dma_start(kT[:, :, 0:128], dws["k"], transpose=True)
                nc.sync.dma_start(kT[:, :, 128:192], dws["qk"][64:128, :], transpose=True)
                for r in range(BAND):
                    first = (i == 0 and r == 0)
                    last = (i == NB - 1 and r == BAND - 1)
                    for h in range(NH):
                        hs = slice(h * HD, h * HD + HD)
                        nc.tensor.matmul(g_ps[:, hs], qT[:, r, hs], kT[:, r, hs],
                                         start=first, stop=last,
                                         skip_group_check=True)

            prev = None
            for i in range(NB):
                cur = pw_band(i)
                if prev is not None:
                    halo_exchange(prev, cur)
                    dw_gram_band(i - 1, prev)
                prev = cur
            for key in BKEYS:
                p3 = prev[key].rearrange("p (h w) -> p h w", w=WP)
                nc.gpsimd.memset(p3[:, BAND + 1:BAND + 2, :], 0.0)
            dw_gram_band(NB - 1, prev)

        # ======================= softmax / attention =======================
        sm_ps = ctx.enter_context(tc.tile_pool(name="sm_ps", bufs=1, space="PSUM"))
        nq2 = small.tile([128, 1], F32, tag="nq2")
        nk2 = small.tile([128, 1], F32, tag="nk2")
        nqk2 = small.tile([128, 1], F32, tag="nqk2")
        for acc, dst in ((n2q0, nq2), (n2k0, nk2), (n2qk1, nqk2)):
            nc.vector.tensor_reduce(dst, acc, axis=mybir.AxisListType.X, op=ALU.add)
            nc.scalar.activation(dst, dst, ACT.Sqrt)
            nc.vector.reciprocal(dst, dst)

        rqh = small.tile([HD, NH], F32, tag="rqh")
        rkh = small.tile([HD, NH], F32, tag="rkh")
        nc.sync.dma_start(rqh[:, 0:1], nq2[0:48, :])
        nc.sync.dma_start(rqh[:, 1:2], nq2[48:96, :])
        nc.sync.dma_start(rqh[0:32, 2:3], nq2[96:128, :])
        nc.sync.dma_start(rqh[32:48, 2:3], nqk2[0:16, :])
        nc.sync.dma_start(rqh[:, 3:4], nqk2[16:64, :])
        nc.sync.dma_start(rkh[:, 0:1], nk2[0:48, :])
        nc.sync.dma_start(rkh[:, 1:2], nk2[48:96, :])
        nc.sync.dma_start(rkh[0:32, 2:3], nk2[96:128, :])
        nc.sync.dma_start(rkh[32:48, 2:3], nqk2[64:80, :])
        nc.sync.dma_start(rkh[:, 3:4], nqk2[80:128, :])

        temp_bc = small.tile([HD, NH], F32, tag="tempbc")
        nc.sync.dma_start(temp_bc, temp_d.to_broadcast([HD, NH]))
        nc.vector.tensor_mul(rqh, rqh, temp_bc)

        g_sb = small.tile([HD, NH * HD], F32, tag="gsb")
        nc.vector.tensor_copy(g_sb, g_ps)
        for h in range(NH):
            hs = slice(h * HD, h * HD + HD)
            nc.vector.tensor_scalar_mul(g_sb[:, hs], g_sb[:, hs], rqh[:, h:h + 1])

        rkT_ps = sm_ps.tile([NH, HD], F32, tag="rkT")
        nc.tensor.transpose(rkT_ps, rkh, ident_f32)
        rkT = small.tile([NH, HD], F32, tag="rkTs")
        nc.vector.tensor_copy(rkT, rkT_ps)
        rk_flat = small.tile([1, NH * HD], F32, tag="rkflat")
        for h in range(NH):
            nc.sync.dma_start(rk_flat[:, h * HD:(h + 1) * HD], rkT[h:h + 1, :])
        ones1 = small.tile([1, HD], F32, tag="ones1")
        nc.vector.memset(ones1, 1.0)
        rk_bc = sm_ps.tile([HD, NH * HD], F32, tag="rkbc")
        nc.tensor.matmul(rk_bc, ones1, rk_flat, start=True, stop=True)
        nc.vector.tensor_mul(g_sb, g_sb, rk_bc)

        # softmax over the k-channel axis per head block
        a_sb = small.tile([HD, NH * HD], F32, tag="asb")
        sexp = small.tile([HD, NH], F32, tag="sexp")
        for h in range(NH):
            hs = slice(h * HD, h * HD + HD)
            mx = small.tile([HD, 1], F32, tag="mx")
            nc.vector.tensor_reduce(mx, g_sb[:, hs], axis=mybir.AxisListType.X,
                                    op=ALU.max)
            nc.vector.tensor_scalar_mul(mx, mx, -1.0)
            nc.scalar.activation(a_sb[:, hs], g_sb[:, hs], ACT.Exp, bias=mx,
                                 accum_out=sexp[:, h:h + 1])
        nc.vector.reciprocal(sexp, sexp)
        for h in range(NH):
            hs = slice(h * HD, h * HD + HD)
            nc.vector.tensor_scalar_mul(a_sb[:, hs], a_sb[:, hs], sexp[:, h:h + 1])

        a_bf = small.tile([HD, NH * HD], BF, tag="abf")
        nc.vector.tensor_copy(a_bf, a_sb)
        bd01 = small.tile([96, 96], BF, tag="bd01")
        bd23 = small.tile([96, 96], BF, tag="bd23")
        for bd, off in ((bd01, 0), (bd23, 96)):
            tps = sm_ps.tile([96, HD], BF, tag="attT")
            nc.tensor.transpose(tps, a_bf[:, off:off + 96], ident_bf)
            tsb = small.tile([96, HD], BF, tag="attTs")
            nc.vector.tensor_copy(tsb, tps)
            nc.vector.memset(bd, 0.0)
            # compute-engine APs must start at partition 0/32/64/96; the
            # 48-offset block placement goes through DMA instead
            nc.vector.tensor_copy(bd[0:48, 0:48], tsb[0:48, :])
            nc.sync.dma_start(bd[48:96, 48:96], tsb[48:96, :])

        # =========================== PHASE B ===========================
        with ExitStack() as ctxb:
            vdwp = ctxb.enter_context(tc.tile_pool(name="vdwp", bufs=2))
            aop = ctxb.enter_context(tc.tile_pool(name="aop", bufs=4))
            psb = ctxb.enter_context(tc.tile_pool(name="psb", bufs=2, space="PSUM"))

            for i in range(NB):
                vda = vdwp.tile([96, BN], BF, tag="vda")
                vdb = vdwp.tile([96, BN], BF, tag="vdb")
                nc.sync.dma_start(vda, vdw_dram_a[:, i * BN:(i + 1) * BN])
                nc.sync.dma_start(vdb, vdw_dram_b[:, i * BN:(i + 1) * BN])
                for j in range(NSL):
                    sl = slice(j * 512, j * 512 + 512)
                    n0 = i * BN + j * 512
                    ao_ps_a = psb.tile([96, 512], F32, tag="ao")
                    ao_ps_b = psb.tile([96, 512], F32, tag="ao")
                    nc.tensor.matmul(ao_ps_a, bd01, vda[:, sl], start=True, stop=True)
                    nc.tensor.matmul(ao_ps_b, bd23, vdb[:, sl], start=True, stop=True)
                    ao_a = aop.tile([96, 512], BF, tag="aoa")
                    ao_b = aop.tile([96, 512], BF, tag="aob")
                    nc.vector.tensor_copy(ao_a, ao_ps_a)
                    nc.vector.tensor_copy(ao_b, ao_ps_b)
                    op = psb.tile([128, 512], F32, tag="wout")
                    nc.tensor.matmul(op, woT_a[:, 0:128], ao_a, start=True, stop=False)
                    nc.tensor.matmul(op, woT_b[:, 0:128], ao_b, start=False, stop=True)
                    oph = psb.tile([64, 512], F32, tag="wout")
                    nc.tensor.matmul(oph, woT_a[:, 128:192], ao_a, start=True, stop=False)
                    nc.tensor.matmul(oph, woT_b[:, 128:192], ao_b, start=False, stop=True)
                    osb = aop.tile([128, 512], F32, tag="osb")
                    osbh = aop.tile([64, 512], F32, tag="osbh")
                    nc.scalar.copy(osb, op)
                    nc.vector.tensor_copy(osbh, oph)
                    nc.scalar.dma_start(out_d[0:128, n0:n0 + 512], osb)
                    nc.scalar.dma_start(out_d[128:192, n0:n0 + 512], osbh)

    nc.compile()
    return nc


def _get_nc():
    if "nc" not in _NC_CACHE:
        _NC_CACHE["nc"] = build_nc()
    return _NC_CACHE["nc"]


def _prep_in_maps(f_opt, f_sar, w_q, w_qdw, w_kv, w_kvdw, w_out, temperature):
    bf = ml_dtypes.bfloat16
    f_opt, f_sar, w_q, w_qdw, w_kv, w_kvdw, w_out, temperature = (
        np.asarray(a) for a in
        (f_opt, f_sar, w_q, w_qdw, w_kv, w_kvdw, w_out, temperature))
    f8 = ml_dtypes.float8_e4m3
    wq = w_q[:, :, 0, 0]
    wk = w_kv[0:C, :, 0, 0]

    def dr_pack(w, cols):
        sel = np.asarray(w, np.float32)[cols, :] * SCL
        arr = sel.T.reshape(2, 96, len(cols)).transpose(1, 0, 2)
        return np.ascontiguousarray(arr.reshape(96, 2 * len(cols))).astype(f8)

    wq8 = dr_pack(wq, range(0, 128))
    wqh8 = dr_pack(wq, range(128, 192))
    wk8 = dr_pack(wk, range(0, 128))
    wkh8 = dr_pack(wk, range(128, 192))
    wv_t = np.ascontiguousarray(w_kv[C:2 * C, :, 0, 0].T).astype(bf)
    wo_t = np.ascontiguousarray(w_out[:, :, 0, 0].T).astype(bf)
    dwq = np.ascontiguousarray(w_qdw.reshape(C, 9)).astype(np.float32)
    dwk = np.ascontiguousarray(w_kvdw[0:C].reshape(C, 9)).astype(np.float32)
    dwv = np.ascontiguousarray(w_kvdw[C:2 * C].reshape(C, 9)).astype(np.float32)
    temp = np.ascontiguousarray(temperature.reshape(1, NH)).astype(np.float32)
    fo = np.asarray(f_opt).reshape(B, C, N).astype(bf)
    fs = np.asarray(f_sar).reshape(B, C, N).astype(bf)
    in_maps = []
    fof = np.asarray(f_opt, np.float32).reshape(B, C, N)
    fsf = np.asarray(f_sar, np.float32).reshape(B, C, N)
    for b in range(B):
        xo8 = np.ascontiguousarray(
            fof[b].reshape(2, 96, N).transpose(1, 0, 2).reshape(96, 2 * N)).astype(f8)
        xr8 = np.ascontiguousarray(
            fsf[b].reshape(2, 96, N).transpose(1, 0, 2).reshape(96, 2 * N)).astype(f8)
        in_maps.append({
            "x_opt": np.ascontiguousarray(fo[b]),
            "x_sar": np.ascontiguousarray(fs[b]),
            "x_opt8": xo8, "x_sar8": xr8,
            "w_q8": wq8, "w_qh8": wqh8, "w_k8": wk8, "w_kh8": wkh8,
            "w_v_t": wv_t, "w_o_t": wo_t,
            "dw_q": dwq, "dw_k": dwk, "dw_v": dwv, "temp": temp,
        })
    return in_maps


def kernel(f_opt, f_sar, w_q, w_qdw, w_kv, w_kvdw, w_out, temperature,
           **run_kwargs):
    nc = _get_nc()
    in_maps = _prep_in_maps(f_opt, f_sar, w_q, w_qdw, w_kv, w_kvdw, w_out,
                            temperature)
    res = run_bass_kernel_spmd(nc, in_maps, core_ids=list(range(B)), **run_kwargs)
    out = np.stack([res.results[b]["out"].reshape(C, HH, WW) for b in range(B)])
    if run_kwargs:
        return out.astype(np.float32), res
    return out.astype(np.float32)



# revision 24
# speedup vs baseline: 1.0077x; 1.0077x over previous
"""CrossModalMDTA Trainium2 kernel (8-core data-parallel over batch).

Per-core pipeline (one batch sample, C=192, H=W=128, 4 heads, head_dim=48):
  q  = dw3x3(conv1x1(f_opt, w_q), w_qdw)            [C, N]
  kv = dw3x3(conv1x1(f_sar, w_kv), w_kvdw)          [2C, N]
  G  = (q/|q|) @ (k/|k|)^T per head  (48x48)        l2 norms applied to the
                                                    Gram matrix, not the tensors
  out = w_out @ (softmax(G*temp) @ v)               [C, N]

Layout: channels on partitions, flattened spatial on the free dim with a
4-column pad per image row ([C, 128, 132], valid w in 2..130) so the nine
depthwise taps are free-dim-shifted multiply-accumulates with correct zero
padding.  1x1 convs / attention / output projection run on TensorE in bf16
with fp32 PSUM accumulation.  The depthwise conv is split between TensorE
(diag-weight matmuls whose shifted reads come from the rhs access pattern,
accumulated in PSUM) and VectorE (tensor_scalar product at 4x + tensor_tensor
add at 2x).  The Gram contraction gets its [N, C] operands from batched xbar
DMA transposes (one instruction per band transposes 16 128x128 blocks).
v's depthwise output round-trips through DRAM between phases to fit SBUF.
"""

import numpy as np
import ml_dtypes
from contextlib import ExitStack

import concourse.bass as bass
import concourse.mybir as mybir
import concourse.tile as tile
from concourse import bacc
from concourse.ap import AP
from concourse.bass_utils import run_bass_kernel_spmd
from concourse.masks import make_identity

BF = mybir.dt.bfloat16
F8 = mybir.dt.float8e4
F32 = mybir.dt.float32
DR = mybir.MatmulPerfMode.DoubleRow
SCL = 64.0
ALU = mybir.AluOpType
ACT = mybir.ActivationFunctionType

B = 8
C = 192
HH = 128
WW = 128
NH = 4
HD = 48
N = HH * WW            # 16384
WP = WW + 4            # 132 padded row width (2 guard cols each side)
NP = HH * WP           # 16896
BAND = 16              # h-rows per band
NB = HH // BAND        # 8 bands
BN = BAND * WW         # 2048 valid elems per band
BROWS = BAND + 2       # band buffer rows (1-row halo each side)
BBUF = BROWS * WP      # 2376
NSL = BN // 512        # 512-wide psum slices per band

# which depthwise blocks go on the PE (diag matmuls) per band; the rest go on
# the DVE (tensor_scalar + tensor_tensor).  Tunable balance knob.  q/k/qk run
# on PE in fp8 DoubleRow tap-pair form (precision-free: softmax logits are
# tiny, so Q/K-path quantization error is attenuated to nothing).
PE_DW = {
    "va": [True, False, True, False, True, False, True, False],
    "vb": [False, True, False, True, False, True, False, True],
}

# fp8-DR tap pairing: two (dh, dw) taps per matmul; lone center rides a
# stride-0 second row with zero weight.
DR_PAIRS = [((-1, -1), (-1, 0)), ((-1, 1), (0, -1)), ((0, 1), (1, -1)),
            ((1, 0), (1, 1)), ((0, 0), None)]
DR_PAIR_T = [(0, 1), (2, 3), (5, 6), (7, 8), (4, None)]  # tap indices, row-major
SCLW = 64.0  # fp8 scale for depthwise diag weights

_NC_CACHE = {}


def _taps():
    # (tap index, dh, dw) for the 3x3 correlation; center first so it can
    # initialize the accumulator (never range-restricted, always full-size).
    out = [(4, 0, 0)]
    for ky in range(3):
        for kx in range(3):
            t = ky * 3 + kx
            if t != 4:
                out.append((t, ky - 1, kx - 1))
    return out


def build_nc():
    nc = bacc.Bacc("TRN2", target_bir_lowering=False, debug=False, num_devices=B)

    xo_d = nc.dram_tensor("x_opt", [C, N], BF, kind="ExternalInput").ap()
    xs_d = nc.dram_tensor("x_sar", [C, N], BF, kind="ExternalInput").ap()
    xo8_d = nc.dram_tensor("x_opt8", [96, 2 * N], F8, kind="ExternalInput").ap()
    xr8_d = nc.dram_tensor("x_sar8", [96, 2 * N], F8, kind="ExternalInput").ap()
    wq8_d = nc.dram_tensor("w_q8", [96, 2 * 128], F8, kind="ExternalInput").ap()
    wqh8_d = nc.dram_tensor("w_qh8", [96, 2 * 64], F8, kind="ExternalInput").ap()
    wk8_d = nc.dram_tensor("w_k8", [96, 2 * 128], F8, kind="ExternalInput").ap()
    wkh8_d = nc.dram_tensor("w_kh8", [96, 2 * 64], F8, kind="ExternalInput").ap()
    wvT_d = nc.dram_tensor("w_v_t", [C, C], BF, kind="ExternalInput").ap()
    woT_d = nc.dram_tensor("w_o_t", [C, C], BF, kind="ExternalInput").ap()
    dwv_d = nc.dram_tensor("dw_v", [C, 9], F32, kind="ExternalInput").ap()
    dgq8_d = nc.dram_tensor("dg_q8", [128, 1280], F8, kind="ExternalInput").ap()
    dgk8_d = nc.dram_tensor("dg_k8", [128, 1280], F8, kind="ExternalInput").ap()
    dgqk8_d = nc.dram_tensor("dg_qk8", [128, 1280], F8, kind="ExternalInput").ap()
    temp_d = nc.dram_tensor("temp", [1, NH], F32, kind="ExternalInput").ap()
    out_d = nc.dram_tensor("out", [C, N], F32, kind="ExternalOutput").ap()

    with ExitStack() as ctx:
        tc = ctx.enter_context(tile.TileContext(nc))
        consts = ctx.enter_context(tc.tile_pool(name="consts", bufs=1))
        small = ctx.enter_context(tc.tile_pool(name="small", bufs=1))
        gram_ps = ctx.enter_context(tc.tile_pool(name="gram_ps", bufs=1, space="PSUM"))
        dram = ctx.enter_context(tc.tile_pool(name="dram", bufs=1, space="DRAM"))

        # ---- weights ----
        wq8 = consts.tile([96, 2, 128], F8, tag="wq8")
        wqh8 = consts.tile([96, 2, 64], F8, tag="wqh8")
        wk8 = consts.tile([96, 2, 128], F8, tag="wk8")
        wkh8 = consts.tile([96, 2, 64], F8, tag="wkh8")
        nc.sync.dma_start(wq8, wq8_d.rearrange("k (j m) -> k j m", j=2))
        nc.sync.dma_start(wqh8, wqh8_d.rearrange("k (j m) -> k j m", j=2))
        nc.sync.dma_start(wk8, wk8_d.rearrange("k (j m) -> k j m", j=2))
        nc.sync.dma_start(wkh8, wkh8_d.rearrange("k (j m) -> k j m", j=2))
        wvT_a = consts.tile([128, C], BF, tag="wva")
        wvT_b = consts.tile([64, C], BF, tag="wvb")
        woT_a = consts.tile([96, C], BF, tag="woa")
        woT_b = consts.tile([96, C], BF, tag="wob")
        nc.sync.dma_start(wvT_a, wvT_d[0:128, :])
        nc.sync.dma_start(wvT_b, wvT_d[128:192, :])
        nc.sync.dma_start(woT_a, woT_d[0:96, :])
        nc.sync.dma_start(woT_b, woT_d[96:192, :])

        # depthwise weights as per-partition scalars (fp32 for DVE scalar ops,
        # bf16 for building the PE diag matrices)
        dwva = consts.tile([96, 9], F32, tag="dwva")       # v channels 0:96
        dwvb = consts.tile([96, 9], F32, tag="dwvb")       # v channels 96:192
        nc.sync.dma_start(dwva, dwv_d[0:96, :])
        nc.sync.dma_start(dwvb, dwv_d[96:192, :])

        # fp8 DoubleRow diag tap-pair weights for q / k / qk chunks
        dg_dr = {}
        for kname, src_d in (("q", dgq8_d), ("k", dgk8_d), ("qk", dgqk8_d)):
            t = consts.tile([128, 5, 2, 128], F8, tag=f"dgdr_{kname}")
            nc.sync.dma_start(t, src_d.rearrange("p (i j m) -> p i j m", i=5, j=2))
            dg_dr[kname] = t

        ident_bf = consts.tile([HD, HD], BF, tag="idbf")
        make_identity(nc, ident_bf)
        ident_f32 = consts.tile([HD, HD], F32, tag="idf32")
        make_identity(nc, ident_f32)
        id96 = consts.tile([96, 96], BF, tag="id96")
        make_identity(nc, id96)

        # PE depthwise diag matrices for va/vb (bf16): diag(w_t) = I * w[:, t]
        dgs_all = {}
        for kname, wsrc, idm, pp in (("va", dwva, id96, 96),
                                     ("vb", dwvb, id96, 96)):
            lst = []
            for t in range(9):
                d = consts.tile([pp, pp], BF, tag=f"dg{kname}{t}")
                nc.vector.tensor_scalar_mul(d, idm, wsrc[:, t:t + 1])
                lst.append(d)
            dgs_all[kname] = lst

        # norm^2 accumulators (one column per band)
        n2q0 = small.tile([128, NB], F32, tag="n2q0")
        n2k0 = small.tile([128, NB], F32, tag="n2k0")
        n2qk1 = small.tile([128, NB], F32, tag="n2qk1")

        g_ps = gram_ps.tile([HD, NH * HD], F32, tag="gps")

        vdw_dram_a = dram.tile([96, N], BF, tag="vdwa")
        vdw_dram_b = dram.tile([96, N], BF, tag="vdwb")

        # =========================== PHASE A ===========================
        with ExitStack() as ctxa:
            xband = ctxa.enter_context(tc.tile_pool(name="xband", bufs=2))
            pwband = ctxa.enter_context(tc.tile_pool(name="pwband", bufs=2))
            xsband = ctxa.enter_context(tc.tile_pool(name="xsband", bufs=2))
            dwband = ctxa.enter_context(tc.tile_pool(name="dwband", bufs=3))
            qtp = ctxa.enter_context(tc.tile_pool(name="qtp", bufs=2))
            sinkp = ctxa.enter_context(tc.tile_pool(name="sinkp", bufs=2))
            ps = ctxa.enter_context(tc.tile_pool(name="ps", bufs=3, space="PSUM"))
            psd = ctxa.enter_context(tc.tile_pool(name="psd", bufs=4, space="PSUM"))

            BKEYS = ("q", "k", "qk", "va", "vb")

            def pw_band(i):
                n0 = i * BN
                xr0 = xband.tile([128, BN], BF, tag="xr0")
                xr1 = xband.tile([64, BN], BF, tag="x1")
                xo8 = xband.tile([96, 2, BN], F8, tag="xo8")
                xr8 = xband.tile([96, 2, BN], F8, tag="xr8")
                nc.gpsimd.dma_start(xr0, xs_d[0:128, n0:n0 + BN])
                nc.gpsimd.dma_start(xr1, xs_d[128:192, n0:n0 + BN])
                for j in range(2):
                    nc.gpsimd.dma_start(xo8[:, j, :],
                                        xo8_d[:, j * N + n0: j * N + n0 + BN])
                    nc.gpsimd.dma_start(xr8[:, j, :],
                                        xr8_d[:, j * N + n0: j * N + n0 + BN])

                tiles = {}
                for key in BKEYS:
                    p = 128 if key in ("q", "k", "qk") else 96
                    dt = F8 if key in ("q", "k", "qk") else BF
                    t = pwband.tile([p, BBUF], dt, tag=f"pw_{key}")
                    tiles[key] = t
                    t3 = t.rearrange("p (h w) -> p h w", w=WP)
                    nc.gpsimd.memset(t3[:, :, 0:2], 0.0)
                    nc.gpsimd.memset(t3[:, :, 130:132], 0.0)
                    if i == 0:
                        nc.gpsimd.memset(t3[:, 0:1, :], 0.0)

                for j in range(NSL):
                    sl = slice(j * 512, j * 512 + 512)
                    r0 = 1 + 4 * j          # band-buffer row of this psum slice

                    mm = [
                        ("q", 128, "dr", wq8, xo8),
                        ("qk", 64, "dr", wqh8, xo8),
                        ("k", 128, "dr", wk8, xr8),
                        ("qk2", 64, "dr", wkh8, xr8),
                        ("va", 96, "bf", wvT_a[:, 0:96], wvT_b[:, 0:96]),
                        ("vb", 96, "bf", wvT_a[:, 96:192], wvT_b[:, 96:192]),
                    ]
                    for name, pp, kind, la, lb in mm:
                        pt = ps.tile([pp, 512], F32, tag="pw")
                        if kind == "dr":
                            nc.tensor.matmul(pt, la, lb[:, :, sl], start=True,
                                             stop=True, perf_mode=DR)
                        else:
                            nc.tensor.matmul(pt, la, xr0[:, sl], start=True, stop=False)
                            nc.tensor.matmul(pt, lb, xr1[:, sl], start=False, stop=True)
                        pview = pt.rearrange("p (r w) -> p r w", w=WW)
                        if name == "qk":
                            dst = tiles["qk"].rearrange("p (h w) -> p h w", w=WP)
                            nc.scalar.copy(dst[0:64, r0:r0 + 4, 2:130], pview)
                        elif name == "qk2":
                            dst = tiles["qk"].rearrange("p (h w) -> p h w", w=WP)
                            nc.scalar.copy(dst[64:128, r0:r0 + 4, 2:130], pview)
                        else:
                            dst = tiles[name].rearrange("p (h w) -> p h w", w=WP)
                            nc.scalar.copy(dst[:, r0:r0 + 4, 2:130], pview)
                return tiles

            def halo_exchange(prev, cur):
                # prev row 17 <- cur row 1 ; cur row 0 <- prev row 16
                for key in BKEYS:
                    p3 = prev[key].rearrange("p (h w) -> p h w", w=WP)
                    c3 = cur[key].rearrange("p (h w) -> p h w", w=WP)
                    nc.vector.tensor_copy(p3[:, BAND + 1:BAND + 2, :], c3[:, 1:2, :])
                    nc.vector.tensor_copy(c3[:, 0:1, :], p3[:, BAND:BAND + 1, :])

            def dw_dve(src, wtile, dst, parts):
                # tensor_scalar product (4x) + tensor_tensor add (2x)
                xs = xsband.tile([parts, BBUF], BF, tag="xs")
                nc.vector.tensor_copy(xs[:, 0:BBUF - 2], src[:, 1:BBUF - 1])
                dst3 = dst.rearrange("p (r w) -> p r w", w=WW)
                s3 = src.rearrange("p (h w) -> p h w", w=WP)
                x3 = xs.rearrange("p (h w) -> p h w", w=WP)
                for t, dh, dw in _taps():
                    br = 1 + dh
                    if dw == 0:
                        insl = s3[:, br:br + BAND, 2:130]
                    elif dw == 1:
                        insl = x3[:, br:br + BAND, 2:130]
                    else:
                        insl = x3[:, br:br + BAND, 0:128]
                    if t == 4:
                        nc.vector.tensor_scalar_mul(dst3, insl, wtile[:, t:t + 1])
                    else:
                        p = sinkp.tile([parts, BAND * WW], BF, tag="prod")
                        p3 = p.rearrange("p (r w) -> p r w", w=WW)
                        nc.vector.tensor_scalar_mul(p3, insl, wtile[:, t:t + 1])
                        nc.vector.tensor_add(dst, dst, p)
                return xs

            def dw_pe(src, dgs, dst, parts):
                # diag(w_t) matmuls, shifts via the rhs access pattern,
                # accumulated in PSUM; center tap first (start=True)
                s3 = src.rearrange("p (h w) -> p h w", w=WP)
                for j in range(NSL):
                    pt = psd.tile([parts, 512], F32, tag="dw")
                    r0 = 1 + 4 * j
                    for t, dh, dw in _taps():
                        br = r0 + dh
                        if dw == 0:
                            rhs = s3[:, br:br + 4, 2:130]
                        elif dw == 1:
                            rhs = s3[:, br:br + 4, 3:131]
                        else:
                            rhs = s3[:, br:br + 4, 1:129]
                        nc.tensor.matmul(pt, dgs[t], rhs, start=(t == 4),
                                         stop=(t == 8), skip_group_check=True)
                    nc.scalar.copy(dst[:, j * 512:(j + 1) * 512], pt)

            def dw_pe_dr(src, dg, dst, parts, evac):
                # fp8 DoubleRow: two diag taps per matmul; the pair's second
                # tap rides the rhs AP's extra stride-2 dim.
                pstride = src.ap[0][0]
                for j in range(NSL):
                    pt = psd.tile([parts, 512], F32, tag="dw")
                    r0 = 1 + 4 * j
                    for i, (t1, t2) in enumerate(DR_PAIRS):
                        base1 = (r0 + t1[0]) * WP + 2 + t1[1]
                        d = 0 if t2 is None else \
                            (t2[0] - t1[0]) * WP + (t2[1] - t1[1])
                        rhs = AP(src.tensor, src.offset + base1,
                                 [(pstride, parts), (d, 2), (WP, 4), (1, 128)])
                        nc.tensor.matmul(pt, dg[:, i, :, :], rhs,
                                         start=(i == 0), stop=(i == 4),
                                         perf_mode=DR, skip_group_check=True)
                    evac(dst[:, j * 512:(j + 1) * 512], pt)

            def dw_gram_band(i, tiles):
                dws = {}
                sinks = {}
                for key, evac in (("q", lambda d, p: nc.vector.tensor_copy(d, p)),
                                  ("k", lambda d, p: nc.vector.tensor_copy(d, p)),
                                  ("qk", lambda d, p: nc.scalar.copy(d, p))):
                    dst = dwband.tile([128, BN], BF, tag=f"dw_{key}")
                    dw_pe_dr(tiles[key], dg_dr[key], dst, 128, evac)
                    dws[key] = dst
                for key, wf, parts in (("va", dwva, 96), ("vb", dwvb, 96)):
                    dst = dwband.tile([parts, BN], BF, tag=f"dw_{key}")
                    if PE_DW[key][i]:
                        dw_pe(tiles[key], dgs_all[key], dst, parts)
                    else:
                        sinks[key] = dw_dve(tiles[key], wf, dst, parts)
                    dws[key] = dst

                # spill v depthwise output to DRAM for phase B
                nc.gpsimd.dma_start(vdw_dram_a[:, i * BN:(i + 1) * BN], dws["va"])
                nc.gpsimd.dma_start(vdw_dram_b[:, i * BN:(i + 1) * BN], dws["vb"])

                # channel norms (sum of squares) for q and k
                for key, acc in (("q", n2q0), ("k", n2k0), ("qk", n2qk1)):
                    sink = sinks.get(key)
                    if sink is None:
                        sink = sinkp.tile([128, BN], BF, tag="nsink")
                    else:
                        sink = sink[:, 0:BN]
                    nc.scalar.activation(sink, dws[key], ACT.Square,
                                         accum_out=acc[:, i:i + 1])

                # batched transposes: one inst flips 16 128x128 blocks
                qT = qtp.tile([128, BAND, C], BF, tag="qT")
                kT = qtp.tile([128, BAND, C], BF, tag="kT")
                nc.sync.dma_start(qT[:, :, 0:128], dws["q"], transpose=True)
                nc.sync.dma_start(qT[:, :, 128:192], dws["qk"][0:64, :], transpose=True)
                nc.sync.dma_start(kT[:, :, 0:128], dws["k"], transpose=True)
                nc.sync.dma_start(kT[:, :, 128:192], dws["qk"][64:128, :], transpose=True)
                for r in range(BAND):
                    first = (i == 0 and r == 0)
                    last = (i == NB - 1 and r == BAND - 1)
                    for h in range(NH):
                        hs = slice(h * HD, h * HD + HD)
                        nc.tensor.matmul(g_ps[:, hs], qT[:, r, hs], kT[:, r, hs],
                                         start=first, stop=last,
                                         skip_group_check=True)

            prev = None
            for i in range(NB):
                cur = pw_band(i)
                if prev is not None:
                    halo_exchange(prev, cur)
                    dw_gram_band(i - 1, prev)
                prev = cur
            for key in BKEYS:
                p3 = prev[key].rearrange("p (h w) -> p h w", w=WP)
                nc.gpsimd.memset(p3[:, BAND + 1:BAND + 2, :], 0.0)
            dw_gram_band(NB - 1, prev)

        # ======================= softmax / attention =======================
        sm_ps = ctx.enter_context(tc.tile_pool(name="sm_ps", bufs=1, space="PSUM"))
        nq2 = small.tile([128, 1], F32, tag="nq2")
        nk2 = small.tile([128, 1], F32, tag="nk2")
        nqk2 = small.tile([128, 1], F32, tag="nqk2")
        for acc, dst in ((n2q0, nq2), (n2k0, nk2), (n2qk1, nqk2)):
            nc.vector.tensor_reduce(dst, acc, axis=mybir.AxisListType.X, op=ALU.add)
            nc.scalar.activation(dst, dst, ACT.Sqrt)
            nc.vector.reciprocal(dst, dst)

        rqh = small.tile([HD, NH], F32, tag="rqh")
        rkh = small.tile([HD, NH], F32, tag="rkh")
        nc.sync.dma_start(rqh[:, 0:1], nq2[0:48, :])
        nc.sync.dma_start(rqh[:, 1:2], nq2[48:96, :])
        nc.sync.dma_start(rqh[0:32, 2:3], nq2[96:128, :])
        nc.sync.dma_start(rqh[32:48, 2:3], nqk2[0:16, :])
        nc.sync.dma_start(rqh[:, 3:4], nqk2[16:64, :])
        nc.sync.dma_start(rkh[:, 0:1], nk2[0:48, :])
        nc.sync.dma_start(rkh[:, 1:2], nk2[48:96, :])
        nc.sync.dma_start(rkh[0:32, 2:3], nk2[96:128, :])
        nc.sync.dma_start(rkh[32:48, 2:3], nqk2[64:80, :])
        nc.sync.dma_start(rkh[:, 3:4], nqk2[80:128, :])

        temp_bc = small.tile([HD, NH], F32, tag="tempbc")
        nc.sync.dma_start(temp_bc, temp_d.to_broadcast([HD, NH]))
        nc.vector.tensor_mul(rqh, rqh, temp_bc)

        g_sb = small.tile([HD, NH * HD], F32, tag="gsb")
        nc.vector.tensor_copy(g_sb, g_ps)
        for h in range(NH):
            hs = slice(h * HD, h * HD + HD)
            nc.vector.tensor_scalar_mul(g_sb[:, hs], g_sb[:, hs], rqh[:, h:h + 1])

        rkT_ps = sm_ps.tile([NH, HD], F32, tag="rkT")
        nc.tensor.transpose(rkT_ps, rkh, ident_f32)
        rkT = small.tile([NH, HD], F32, tag="rkTs")
        nc.vector.tensor_copy(rkT, rkT_ps)
        rk_flat = small.tile([1, NH * HD], F32, tag="rkflat")
        for h in range(NH):
            nc.sync.dma_start(rk_flat[:, h * HD:(h + 1) * HD], rkT[h:h + 1, :])
        ones1 = small.tile([1, HD], F32, tag="ones1")
        nc.vector.memset(ones1, 1.0)
        rk_bc = sm_ps.tile([HD, NH * HD], F32, tag="rkbc")
        nc.tensor.matmul(rk_bc, ones1, rk_flat, start=True, stop=True)
        nc.vector.tensor_mul(g_sb, g_sb, rk_bc)

        # softmax over the k-channel axis per head block
        a_sb = small.tile([HD, NH * HD], F32, tag="asb")
        sexp = small.tile([HD, NH], F32, tag="sexp")
        for h in range(NH):
            hs = slice(h * HD, h * HD + HD)
            mx = small.tile([HD, 1], F32, tag="mx")
            nc.vector.tensor_reduce(mx, g_sb[:, hs], axis=mybir.AxisListType.X,
                                    op=ALU.max)
            nc.vector.tensor_scalar_mul(mx, mx, -1.0)
            nc.scalar.activation(a_sb[:, hs], g_sb[:, hs], ACT.Exp, bias=mx,
                                 accum_out=sexp[:, h:h + 1])
        nc.vector.reciprocal(sexp, sexp)
        for h in range(NH):
            hs = slice(h * HD, h * HD + HD)
            nc.vector.tensor_scalar_mul(a_sb[:, hs], a_sb[:, hs], sexp[:, h:h + 1])

        a_bf = small.tile([HD, NH * HD], BF, tag="abf")
        nc.vector.tensor_copy(a_bf, a_sb)
        bd01 = small.tile([96, 96], BF, tag="bd01")
        bd23 = small.tile([96, 96], BF, tag="bd23")
        for bd, off in ((bd01, 0), (bd23, 96)):
            tps = sm_ps.tile([96, HD], BF, tag="attT")
            nc.tensor.transpose(tps, a_bf[:, off:off + 96], ident_bf)
            tsb = small.tile([96, HD], BF, tag="attTs")
            nc.vector.tensor_copy(tsb, tps)
            nc.vector.memset(bd, 0.0)
            # compute-engine APs must start at partition 0/32/64/96; the
            # 48-offset block placement goes through DMA instead
            nc.vector.tensor_copy(bd[0:48, 0:48], tsb[0:48, :])
            nc.sync.dma_start(bd[48:96, 48:96], tsb[48:96, :])

        # =========================== PHASE B ===========================
        with ExitStack() as ctxb:
            vdwp = ctxb.enter_context(tc.tile_pool(name="vdwp", bufs=2))
            aop = ctxb.enter_context(tc.tile_pool(name="aop", bufs=6))
            psb = ctxb.enter_context(tc.tile_pool(name="psb", bufs=2, space="PSUM"))

            for i in range(NB):
                vda = vdwp.tile([96, BN], BF, tag="vda")
                vdb = vdwp.tile([96, BN], BF, tag="vdb")
                nc.sync.dma_start(vda, vdw_dram_a[:, i * BN:(i + 1) * BN])
                nc.sync.dma_start(vdb, vdw_dram_b[:, i * BN:(i + 1) * BN])
                for j in range(NSL):
                    sl = slice(j * 512, j * 512 + 512)
                    n0 = i * BN + j * 512
                    ao_ps_a = psb.tile([96, 512], F32, tag="ao")
                    ao_ps_b = psb.tile([96, 512], F32, tag="ao")
                    nc.tensor.matmul(ao_ps_a, bd01, vda[:, sl], start=True, stop=True)
                    nc.tensor.matmul(ao_ps_b, bd23, vdb[:, sl], start=True, stop=True)
                    ao_a = aop.tile([96, 512], BF, tag="aoa")
                    ao_b = aop.tile([96, 512], BF, tag="aob")
                    nc.vector.tensor_copy(ao_a, ao_ps_a)
                    nc.vector.tensor_copy(ao_b, ao_ps_b)
                    op = psb.tile([128, 512], F32, tag="wout")
                    nc.tensor.matmul(op, woT_a[:, 0:128], ao_a, start=True, stop=False)
                    nc.tensor.matmul(op, woT_b[:, 0:128], ao_b, start=False, stop=True)
                    oph = psb.tile([64, 512], F32, tag="wout")
                    nc.tensor.matmul(oph, woT_a[:, 128:192], ao_a, start=True, stop=False)
                    nc.tensor.matmul(oph, woT_b[:, 128:192], ao_b, start=False, stop=True)
                    osb = aop.tile([128, 512], F32, tag="osb")
                    osbh = aop.tile([64, 512], F32, tag="osbh")
                    nc.scalar.copy(osb, op)
                    nc.vector.tensor_copy(osbh, oph)
                    nc.scalar.dma_start(out_d[0:128, n0:n0 + 512], osb)
                    nc.scalar.dma_start(out_d[128:192, n0:n0 + 512], osbh)

    nc.compile()
    return nc


def _get_nc():
    if "nc" not in _NC_CACHE:
        _NC_CACHE["nc"] = build_nc()
    return _NC_CACHE["nc"]


def _prep_in_maps(f_opt, f_sar, w_q, w_qdw, w_kv, w_kvdw, w_out, temperature):
    bf = ml_dtypes.bfloat16
    f_opt, f_sar, w_q, w_qdw, w_kv, w_kvdw, w_out, temperature = (
        np.asarray(a) for a in
        (f_opt, f_sar, w_q, w_qdw, w_kv, w_kvdw, w_out, temperature))
    f8 = ml_dtypes.float8_e4m3
    wq = w_q[:, :, 0, 0]
    wk = w_kv[0:C, :, 0, 0]

    def dr_pack(w, cols):
        sel = np.asarray(w, np.float32)[cols, :] * SCL
        arr = sel.T.reshape(2, 96, len(cols)).transpose(1, 0, 2)
        return np.ascontiguousarray(arr.reshape(96, 2 * len(cols))).astype(f8)

    wq8 = dr_pack(wq, range(0, 128))
    wqh8 = dr_pack(wq, range(128, 192))
    wk8 = dr_pack(wk, range(0, 128))
    wkh8 = dr_pack(wk, range(128, 192))
    wv_t = np.ascontiguousarray(w_kv[C:2 * C, :, 0, 0].T).astype(bf)
    wo_t = np.ascontiguousarray(w_out[:, :, 0, 0].T).astype(bf)
    dwq = np.ascontiguousarray(w_qdw.reshape(C, 9)).astype(np.float32)
    dwk = np.ascontiguousarray(w_kvdw[0:C].reshape(C, 9)).astype(np.float32)
    dwv = np.ascontiguousarray(w_kvdw[C:2 * C].reshape(C, 9)).astype(np.float32)

    def dg_pack(wchunk):
        # [128, 9] taps -> [128, 5 pairs, 2, 128] fp8 diag matrices
        arr = np.zeros((128, 5, 2, 128), np.float32)
        idx = np.arange(128)
        for i, (ta, tb) in enumerate(DR_PAIR_T):
            for j, t in enumerate((ta, tb)):
                if t is None:
                    continue
                arr[idx, i, j, idx] = wchunk[:, t] * SCLW
        return np.ascontiguousarray(arr.reshape(128, 1280)).astype(f8)

    dgq8 = dg_pack(dwq[0:128])
    dgk8 = dg_pack(dwk[0:128])
    dgqk8 = dg_pack(np.concatenate([dwq[128:192], dwk[128:192]], axis=0))
    temp = np.ascontiguousarray(temperature.reshape(1, NH)).astype(np.float32)
    fo = np.asarray(f_opt).reshape(B, C, N).astype(bf)
    fs = np.asarray(f_sar).reshape(B, C, N).astype(bf)
    in_maps = []
    fof = np.asarray(f_opt, np.float32).reshape(B, C, N)
    fsf = np.asarray(f_sar, np.float32).reshape(B, C, N)
    for b in range(B):
        xo8 = np.ascontiguousarray(
            fof[b].reshape(2, 96, N).transpose(1, 0, 2).reshape(96, 2 * N)).astype(f8)
        xr8 = np.ascontiguousarray(
            fsf[b].reshape(2, 96, N).transpose(1, 0, 2).reshape(96, 2 * N)).astype(f8)
        in_maps.append({
            "x_opt": np.ascontiguousarray(fo[b]),
            "x_sar": np.ascontiguousarray(fs[b]),
            "x_opt8": xo8, "x_sar8": xr8,
            "w_q8": wq8, "w_qh8": wqh8, "w_k8": wk8, "w_kh8": wkh8,
            "w_v_t": wv_t, "w_o_t": wo_t, "dw_v": dwv, "temp": temp,
            "dg_q8": dgq8, "dg_k8": dgk8, "dg_qk8": dgqk8,
        })
    return in_maps


def kernel(f_opt, f_sar, w_q, w_qdw, w_kv, w_kvdw, w_out, temperature,
           **run_kwargs):
    nc = _get_nc()
    in_maps = _prep_in_maps(f_opt, f_sar, w_q, w_qdw, w_kv, w_kvdw, w_out,
                            temperature)
    res = run_bass_kernel_spmd(nc, in_maps, core_ids=list(range(B)), **run_kwargs)
    out = np.stack([res.results[b]["out"].reshape(C, HH, WW) for b in range(B)])
    if run_kwargs:
        return out.astype(np.float32), res
    return out.astype(np.float32)

